# revision 23
# baseline (speedup 1.0000x reference)
"""Causal self-attention with Music-Transformer relative position, on 8 TRN2 cores.

Self-contained: takes FULL inputs, shards internally, returns FULL output.

Sharding: core c -> batch b = c // 4, heads [4*(c%4), 4*(c%4)+4)  (2 head-pairs).
Each core computes its qkv column-slice, attention for its 4 heads, and a
row-parallel partial of the output projection. Host sums the 8 partials.

Key tricks:
- Skew: Srel[i, j] = QEr[i, L-1-i+j], so with a padded row-major DRAM buffer
  EQr[L, LP] (LP = L+512, pad = 0) holding exp(QEr/8), a single strided DMA
  (row stride LP-1) yields skew(exp(QEr/8)); exp commutes with the skew:
      P = exp((QK + Srel)/8) = exp(QK/8) * skew(exp(QEr/8))
  and the zero pad makes the causal mask free.
- P^T for the P@V matmul comes from wide xbar DMA-transposes ([128, 512] ->
  [128, 4, 128]) whose j-interleave is whatever the hardware produces —
  v is pre-transposed with the *same* primitive, so both operands of the
  contraction share one permutation of j and the matmul result is unchanged.
- Row-paired K=64 matmuls (two heads on PE row-groups (0,0)/(64,0)).
"""

import numpy as np
from contextlib import ExitStack

import concourse.bass as bass
import concourse.tile as tile
from concourse import mybir, bacc
from concourse.bass_utils import run_bass_kernel_spmd

F32 = mybir.dt.float32
F32R = mybir.dt.float32r
F16 = mybir.dt.float16

B, L, D = 2, 2048, 1024
NH, HS = 16, 64
BLOCK_SIZE = 2048
SCALE = 1.0 / 8.0  # 1/sqrt(HS)
LP = L + 512       # padded EQr row length
N_CORES = 8
HPC = 4            # heads per core (2 pairs)

EXP = mybir.ActivationFunctionType.Exp
COPY = mybir.ActivationFunctionType.Copy
MULT = mybir.AluOpType.mult


def _build_program():
    nc = bacc.Bacc("TRN2", target_bir_lowering=False, debug=False)

    # ---- per-core inputs (f32r dtypes map to np.float32 on the host side) ----
    xT_d = nc.dram_tensor("xT", [D, L], F16, kind="ExternalInput")
    wq_d = nc.dram_tensor("wq", [2, D, 128], F16, kind="ExternalInput")
    wk_d = nc.dram_tensor("wk", [2, D, 128], F16, kind="ExternalInput")
    wv_d = nc.dram_tensor("wv", [2, D, 128], F16, kind="ExternalInput")
    bq_d = nc.dram_tensor("bq", [2, 128, 1], F32, kind="ExternalInput")
    bk_d = nc.dram_tensor("bk", [2, 128, 1], F32, kind="ExternalInput")
    erT_d = nc.dram_tensor("erT", [HS, L], F32R, kind="ExternalInput")
    wproj_d = nc.dram_tensor("wproj", [2, 128, D], F16, kind="ExternalInput")
    out_d = nc.dram_tensor("out", [L, D], F16, kind="ExternalOutput")

    # per-head padded EQr scratch (separate tensors => fine-grained deps)
    eqr_d = [nc.dram_tensor(f"eqr{h}", [L, LP], F16, kind="Internal") for h in range(HPC)]

    with tile.TileContext(nc) as tc, ExitStack() as ctx:
        # ---------- persistent tiles ----------
        persist = ctx.enter_context(tc.tile_pool(name="persist", bufs=1))
        qT = [persist.tile([128, L], F32R, tag=f"qT{p}", name=f"qT{p}") for p in range(2)]
        kT = [persist.tile([128, L], F32R, tag=f"kT{p}", name=f"kT{p}") for p in range(2)]
        # v_perm[p][pp, jb, c, d'] = V[pair p][j = xbar-perm(jb, pp, c), d']
        vperm = [persist.tile([128, 4, 4, 128], F16, tag=f"vperm{p}", name=f"vperm{p}")
                 for p in range(2)]
        yT = [persist.tile([128, L], F16, tag=f"yT{p}", name=f"yT{p}") for p in range(2)]
        wproj_sb = persist.tile([128, 2, D], F16, tag="wproj")
        bq_sb = persist.tile([128, 2], F32, tag="bq")
        bk_sb = persist.tile([128, 2], F32, tag="bk")
        zero16 = persist.tile([128, 512], F16, tag="zero16")
        nc.vector.memset(zero16[:, :], 0.0)
        ident = persist.tile([128, 128], F32, tag="ident")
        from concourse.masks import make_identity
        make_identity(nc, ident[:, :])
        for p in range(2):
            nc.gpsimd.dma_start(wproj_sb[:, p, :], wproj_d.ap()[p])
            nc.gpsimd.dma_start(bq_sb[:, p : p + 1], bq_d.ap()[p])
            nc.gpsimd.dma_start(bk_sb[:, p : p + 1], bk_d.ap()[p])

        # pad region of every eqr
        for h in range(HPC):
            for g in range(16):
                nc.sync.dma_start(eqr_d[h].ap()[g * 128 : (g + 1) * 128, L:LP], zero16[:, :])

        # ---------- phase 1: qkv projection ----------
        with tc.tile_pool(name="ph1", bufs=1) as ph1, \
             tc.tile_pool(name="ph1ps", bufs=3, space="PSUM") as ph1ps, \
             tc.tile_pool(name="ph1tmp", bufs=3) as ph1tmp:
            xT_sb = ph1.tile([128, 8, L], F16)
            nc.gpsimd.dma_start(
                xT_sb[:, :, :],
                xT_d.ap().rearrange("(kc part) i -> part kc i", part=128),
            )
            w_sb = {}
            for name, d_ in (("q", wq_d), ("k", wk_d), ("v", wv_d)):
                t = ph1.tile([128, 2, 8, 128], F16, tag=f"w{name}", name=f"w{name}sb")
                nc.gpsimd.dma_start(
                    t[:, :, :, :],
                    d_.ap().rearrange("pp (kc part) col -> part pp kc col", part=128),
                )
                w_sb[name] = t

            vT = [ph1.tile([128, L], F16, tag=f"vT{p}", name=f"vT{p}") for p in range(2)]
            for p in range(2):
                for ib in range(4):
                    isl = bass.ts(ib, 512)
                    for name in ("q", "k", "v"):
                        ps_t = ph1ps.tile([128, 512], F32, tag="qkv")
                        for kc in range(8):
                            nc.tensor.matmul(
                                ps_t[:, :],
                                lhsT=w_sb[name][:, p, kc, :],
                                rhs=xT_sb[:, kc, isl],
                                start=(kc == 0),
                                stop=(kc == 7),
                            )
                        if name == "q":
                            nc.vector.tensor_scalar_add(qT[p][:, isl], ps_t[:, :], bq_sb[:, p : p + 1])
                        elif name == "k":
                            nc.vector.tensor_scalar_add(kT[p][:, isl], ps_t[:, :], bk_sb[:, p : p + 1])
                        else:
                            nc.vector.tensor_copy(vT[p][:, isl], ps_t[:, :])
            # wide xbar transposes vT -> vperm (same primitive as P^T below)
            for p in range(2):
                for jb in range(4):
                    nc.sync.dma_start_transpose(
                        vperm[p][:, jb, :, :], vT[p][:, bass.ts(jb, 512)]
                    )

        # ---------- phases 2+3 per head-pair ----------
        with tc.tile_pool(name="p23", bufs=1) as p23, \
             tc.tile_pool(name="p2tmp", bufs=2) as p2tmp, \
             tc.tile_pool(name="p3qk0", bufs=3, space="PSUM") as p3qk0, \
             tc.tile_pool(name="p3qk1", bufs=3, space="PSUM") as p3qk1, \
             tc.tile_pool(name="p3y", bufs=1, space="PSUM") as p3y, \
             tc.tile_pool(name="p3pk", bufs=2) as p3pk, \
             tc.tile_pool(name="p3es", bufs=3) as p3es, \
             tc.tile_pool(name="p3pw", bufs=2) as p3pw, \
             tc.tile_pool(name="p3pt", bufs=2) as p3pt, \
             tc.tile_pool(name="p3rs", bufs=2) as p3rs:
            erT_sb = p23.tile([128, L], F32R)
            nc.gpsimd.dma_start(erT_sb[0:HS, :], erT_d.ap())
            nc.gpsimd.dma_start(erT_sb[HS:128, :], erT_d.ap())
            p3qk = [p3qk0, p3qk1]

            pending = []

            def _emit_attv(item):
                pp_, IB_, njb_, ptjb_, rb_ = item
                for s_ in range(2):
                    ps_y = p3y.tile([64, 512], F32, tag=f"y{s_}", name=f"yps{s_}")
                    nmm = 4 * njb_
                    m_ = 0
                    for jb_ in range(njb_):
                        for c_ in range(4):
                            nc.tensor.matmul(
                                ps_y[:, :],
                                lhsT=vperm[pp_][:, jb_, c_, 64 * s_ : 64 * s_ + 64],
                                rhs=ptjb_[(s_, jb_)][:, c_, :],
                                start=(m_ == 0),
                                stop=(m_ == nmm - 1),
                            )
                            m_ += 1
                    nc.vector.scalar_tensor_tensor(
                        out=yT[pp_][64 * s_ : 64 * s_ + 64, bass.ts(IB_, 512)],
                        in0=ps_y[:, :],
                        scalar=1.0,
                        in1=rb_[s_][:, :],
                        op0=mybir.AluOpType.bypass,
                        op1=MULT,
                    )

            for p in range(2):
                for IB in range(4):
                    # ----- phase 2: EQr stripes for this IB (rows [512*IB, 512*IB+512)) -----
                    for ib in range(4 * IB, 4 * IB + 4):
                        i0 = ib * 128
                        mlo = (L - 128 - i0) // 512
                        for s in range(2):
                            h = 2 * p + s
                            sl = slice(64 * s, 64 * s + 64)
                            eq = p2tmp.tile([128, 2048], F16, tag=f"eq{s}", name=f"eq{s}")
                            for mb in range(mlo, 4):
                                ps_t = p3qk[s].tile([128, 512], F32, tag=f"qk{s}", name=f"eqrps{s}")
                                nc.tensor.matmul(
                                    ps_t[:, :],
                                    lhsT=qT[p][sl, i0 : i0 + 128],
                                    rhs=erT_sb[sl, bass.ts(mb, 512)],
                                    start=True,
                                    stop=True,
                                    tile_position=(64 * s, 0),
                                )
                                nc.scalar.activation(
                                    eq[:, bass.ts(mb, 512)],
                                    ps_t[:, :],
                                    EXP,
                                    scale=SCALE,
                                )
                            nc.gpsimd.dma_start(
                                eqr_d[h].ap()[i0 : i0 + 128, mlo * 512 : 2048],
                                eq[:, mlo * 512 : 2048],
                            )
                    # ----- phase 3 for this IB -----
                    njb = IB + 1
                    W = njb * 512
                    ptjb = {}
                    rb = {}
                    for s in range(2):
                        h = 2 * p + s
                        sl = slice(64 * s, 64 * s + 64)
                        rsall = p3rs.tile([128, 4], F32, tag=f"rs{s}", name=f"rs{s}")
                        for ib4 in range(4):
                            i0 = 512 * IB + 128 * ib4
                            pk = p3pk.tile([128, 2048], F16, tag=f"pk{s}", name=f"pk{s}")
                            for jb in range(njb):
                                ps_t = p3qk[s].tile([128, 512], F32, tag=f"qk{s}", name=f"qkps{s}")
                                nc.tensor.matmul(
                                    ps_t[:, :],
                                    lhsT=qT[p][sl, i0 : i0 + 128],
                                    rhs=kT[p][sl, bass.ts(jb, 512)],
                                    start=True,
                                    stop=True,
                                    tile_position=(64 * s, 0),
                                )
                                nc.scalar.activation(
                                    pk[:, bass.ts(jb, 512)],
                                    ps_t[:, :],
                                    EXP,
                                    scale=SCALE,
                                )
                            # skew read: one wide strided DMA
                            es = p3es.tile([128, 2048], F16, tag=f"es{s}", name=f"es{s}")
                            src = bass.AP(
                                tensor=eqr_d[h],
                                offset=(L - 1) + i0 * (LP - 1),
                                ap=[[LP - 1, 128], [1, W]],
                            )
                            nc.gpsimd.dma_start(es[:, 0:W], src)
                            # combine + full row-sum in one DVE op (P stays unnormalized)
                            pw = p3pw.tile([128, 2048], F16, tag=f"pw{s}", name=f"pw{s}")
                            nc.vector.scalar_tensor_tensor(
                                out=pw[:, 0:W],
                                in0=pk[:, 0:W],
                                scalar=1.0,
                                in1=es[:, 0:W],
                                op0=MULT,
                                op1=MULT,
                                accum_out=rsall[:, ib4 : ib4 + 1],
                            )
                            # wide xbar transposes into the interleaved P^T tiles
                            for jb in range(njb):
                                key = (s, jb)
                                if key not in ptjb:
                                    ptjb[key] = p3pt.tile(
                                        [128, 4, 512], F16, tag=f"pt{s}_{jb}", name=f"pt{s}_{jb}"
                                    )
                                nc.sync.dma_start_transpose(
                                    ptjb[key][:, :, ib4 * 128 : ib4 * 128 + 128],
                                    pw[:, bass.ts(jb, 512)],
                                )
                        # 1/rowsum -> [1, i] layout -> broadcast to 64 partitions
                        riall = p3rs.tile([128, 4], F32, tag=f"ri{s}", name=f"ri{s}")
                        nc.vector.reciprocal(riall[:, :], rsall[:, :])
                        rbt = p3rs.tile([64, 512], F32, tag=f"rb{s}", name=f"rb{s}")
                        for ib4 in range(4):
                            rT1 = p3qk[s].tile([1, 128], F32, tag=f"qk{s}", name=f"rT{s}_{ib4}")
                            nc.tensor.transpose(
                                rT1[:, :], riall[:, ib4 : ib4 + 1], ident[:, :]
                            )
                            rsb1 = p3rs.tile([1, 128], F32, tag=f"rsb{s}", name=f"rsb{s}_{ib4}")
                            nc.vector.tensor_copy(rsb1[:, :], rT1[0:1, :])
                            nc.gpsimd.partition_broadcast(
                                rbt[:, bass.ts(ib4, 128)], rsb1[0:1, :]
                            )
                        rb[s] = rbt
                    # software-pipelined att@v: emit for the PREVIOUS group
                    pending.append((p, IB, njb, ptjb, rb))
                    if len(pending) > 1:
                        _emit_attv(pending.pop(0))

            while pending:
                _emit_attv(pending.pop(0))

        # ---------- phase 4: out_partial = y @ Wproj ----------
        with tc.tile_pool(name="p4ps", bufs=3, space="PSUM") as p4ps, \
             tc.tile_pool(name="p4tmp", bufs=3) as p4tmp:
            for i128 in range(16):
                for eb in range(2):
                    ps_o = p4ps.tile([128, 512], F32, tag="proj")
                    for p in range(2):
                        nc.tensor.matmul(
                            ps_o[:, :],
                            lhsT=yT[p][:, bass.ts(i128, 128)],
                            rhs=wproj_sb[:, p, bass.ts(eb, 512)],
                            start=(p == 0),
                            stop=(p == 1),
                        )
                    o_t = p4tmp.tile([128, 512], F16, tag="out")
                    nc.vector.tensor_copy(o_t[:, :], ps_o[:, :])
                    nc.sync.dma_start(
                        out_d.ap()[bass.ts(i128, 128), bass.ts(eb, 512)], o_t[:, :]
                    )

    nc.compile()
    return nc


_NC_CACHE = None
RUN_KWARGS = {}
LAST_RESULTS = None
LAST_IN_MAPS = None


def _get_program():
    global _NC_CACHE
    if _NC_CACHE is None:
        _NC_CACHE = _build_program()
    return _NC_CACHE


def kernel(x, Wqkv, bqkv, Wproj, bproj, Er):
    x = np.asarray(x, dtype=np.float32)
    Wqkv = np.asarray(Wqkv, dtype=np.float32)
    bqkv = np.asarray(bqkv, dtype=np.float32)
    Wproj = np.asarray(Wproj, dtype=np.float32)
    bproj = np.asarray(bproj, dtype=np.float32)
    Er = np.asarray(Er, dtype=np.float32)

    nc = _get_program()

    start = BLOCK_SIZE - L
    erT = np.ascontiguousarray(Er[start:, :].T)  # [HS, L]

    in_maps = []
    for c in range(N_CORES):
        b = c // 4
        h0 = HPC * (c % 4)
        xT = np.ascontiguousarray(x[b].T).astype(np.float16)  # [D, L]
        wq = np.empty((2, D, 128), np.float16)
        wk = np.empty((2, D, 128), np.float16)
        wv = np.empty((2, D, 128), np.float16)
        bq = np.empty((2, 128, 1), np.float32)
        bk = np.empty((2, 128, 1), np.float32)
        wproj = np.empty((2, 128, D), np.float16)
        for p in range(2):
            c0 = (h0 + 2 * p) * HS  # first head dim offset of the pair
            wq[p] = Wqkv[:, c0 : c0 + 128]
            wk[p] = Wqkv[:, D + c0 : D + c0 + 128]
            wv[p] = Wqkv[:, 2 * D + c0 : 2 * D + c0 + 128]
            bq[p, :, 0] = bqkv[c0 : c0 + 128]
            bk[p, :, 0] = bqkv[D + c0 : D + c0 + 128]
            wproj[p] = Wproj[c0 : c0 + 128, :].astype(np.float16)
        in_maps.append(
            {
                "xT": xT,
                "wq": wq,
                "wk": wk,
                "wv": wv,
                "bq": bq,
                "bk": bk,
                "erT": erT,
                "wproj": wproj,
            }
        )

    global LAST_RESULTS, LAST_IN_MAPS
    LAST_IN_MAPS = in_maps
    res = run_bass_kernel_spmd(nc, in_maps, core_ids=list(range(N_CORES)), **RUN_KWARGS)
    LAST_RESULTS = res

    # host gather: sum row-parallel partials per batch; fold bv@Wproj + bproj
    bv = bqkv[2 * D :]
    bias_vec = bv @ Wproj + bproj  # exact: softmax rows sum to 1
    out = np.zeros((B, L, D), np.float32)
    for c in range(N_CORES):
        out[c // 4] += res.results[c]["out"].astype(np.float32)
    out += bias_vec[None, None, :]
    return out


# revision 26
# speedup vs baseline: 2.5839x; 2.5839x over previous
"""Causal self-attention with Music-Transformer relative position, on 8 TRN2 cores.

Self-contained: takes FULL inputs, shards internally, returns FULL output.

Sharding: core c -> batch b = c // 4, heads [4*(c%4), 4*(c%4)+4)  (2 head-pairs).
Each core computes its qkv column-slice, attention for its 4 heads, and a
row-parallel partial of the output projection. Host sums the 8 partials.

Key tricks:
- Skew: Srel[i, j] = QEr[i, L-1-i+j], so with a padded row-major DRAM buffer
  EQr[L, LP] (LP = L+512, pad = 0) holding exp(QEr/8), a single strided DMA
  (row stride LP-1) yields skew(exp(QEr/8)); exp commutes with the skew:
      P = exp((QK + Srel)/8) = exp(QK/8) * skew(exp(QEr/8))
  and the zero pad makes the causal mask free.
- P^T for the P@V matmul comes from wide xbar DMA-transposes ([128, 512] ->
  [128, 4, 128]) whose j-interleave is whatever the hardware produces —
  v is pre-transposed with the *same* primitive, so both operands of the
  contraction share one permutation of j and the matmul result is unchanged.
- Row-paired K=64 matmuls (two heads on PE row-groups (0,0)/(64,0)).
"""

import numpy as np
from contextlib import ExitStack

import concourse.bass as bass
import concourse.tile as tile
from concourse import mybir, bacc
from concourse.bass_utils import run_bass_kernel_spmd

F32 = mybir.dt.float32
F32R = mybir.dt.float32r
F16 = mybir.dt.float16

B, L, D = 2, 2048, 1024
NH, HS = 16, 64
BLOCK_SIZE = 2048
SCALE = 1.0 / 8.0  # 1/sqrt(HS)
LP = L + 512       # padded EQr row length
N_CORES = 8
HPC = 4            # heads per core (2 pairs)

EXP = mybir.ActivationFunctionType.Exp
COPY = mybir.ActivationFunctionType.Copy
MULT = mybir.AluOpType.mult


def _build_program():
    nc = bacc.Bacc("TRN2", target_bir_lowering=False, debug=False)

    # ---- per-core inputs (f32r dtypes map to np.float32 on the host side) ----
    xT_d = nc.dram_tensor("xT", [D, L], F16, kind="ExternalInput")
    wq_d = nc.dram_tensor("wq", [2, D, 128], F16, kind="ExternalInput")
    wk_d = nc.dram_tensor("wk", [2, D, 128], F16, kind="ExternalInput")
    wv_d = nc.dram_tensor("wv", [2, D, 128], F16, kind="ExternalInput")
    bq_d = nc.dram_tensor("bq", [2, 128, 1], F32, kind="ExternalInput")
    bk_d = nc.dram_tensor("bk", [2, 128, 1], F32, kind="ExternalInput")
    erT_d = nc.dram_tensor("erT", [HS, L], F32R, kind="ExternalInput")
    wproj_d = nc.dram_tensor("wproj", [2, 128, D], F16, kind="ExternalInput")
    out_d = nc.dram_tensor("out", [L, D], F16, kind="ExternalOutput")

    # per-head padded EQr scratch (separate tensors => fine-grained deps)
    eqr_d = [nc.dram_tensor(f"eqr{h}", [L, LP], F16, kind="Internal") for h in range(HPC)]

    with tile.TileContext(nc) as tc, ExitStack() as ctx:
        # ---------- persistent tiles ----------
        persist = ctx.enter_context(tc.tile_pool(name="persist", bufs=1))
        qT = [persist.tile([128, L], F32R, tag=f"qT{p}", name=f"qT{p}") for p in range(2)]
        kT = [persist.tile([128, L], F32R, tag=f"kT{p}", name=f"kT{p}") for p in range(2)]
        # v_perm[p][pp, jb, c, d'] = V[pair p][j = xbar-perm(jb, pp, c), d']
        vperm = [persist.tile([128, 4, 4, 128], F16, tag=f"vperm{p}", name=f"vperm{p}")
                 for p in range(2)]
        yT = [persist.tile([128, L], F16, tag=f"yT{p}", name=f"yT{p}") for p in range(2)]
        wproj_sb = persist.tile([128, 2, D], F16, tag="wproj")
        bq_sb = persist.tile([128, 2], F32, tag="bq")
        bk_sb = persist.tile([128, 2], F32, tag="bk")
        zero16 = persist.tile([128, 512], F16, tag="zero16")
        nc.vector.memset(zero16[:, :], 0.0)
        ident = persist.tile([128, 128], F32, tag="ident")
        from concourse.masks import make_identity
        make_identity(nc, ident[:, :])
        for p in range(2):
            nc.gpsimd.dma_start(wproj_sb[:, p, :], wproj_d.ap()[p])
            nc.gpsimd.dma_start(bq_sb[:, p : p + 1], bq_d.ap()[p])
            nc.gpsimd.dma_start(bk_sb[:, p : p + 1], bk_d.ap()[p])

        # pad region of every eqr
        for h in range(HPC):
            for g in range(16):
                nc.sync.dma_start(eqr_d[h].ap()[g * 128 : (g + 1) * 128, L:LP], zero16[:, :])

        # ---------- phase 1: qkv projection ----------
        with tc.tile_pool(name="ph1", bufs=1) as ph1, \
             tc.tile_pool(name="ph1ps", bufs=3, space="PSUM") as ph1ps, \
             tc.tile_pool(name="ph1tmp", bufs=3) as ph1tmp:
            xT_sb = ph1.tile([128, 8, L], F16)
            w_sb = {}
            for name, d_ in (("q", wq_d), ("k", wk_d), ("v", wv_d)):
                w_sb[name] = ph1.tile([128, 2, 8, 128], F16, tag=f"w{name}", name=f"w{name}sb")
            # per-kc loads: fine-grained deps let the first matmuls start early
            for kc in range(8):
                for name, d_ in (("q", wq_d), ("k", wk_d), ("v", wv_d)):
                    nc.gpsimd.dma_start(
                        w_sb[name][:, :, kc, :],
                        d_.ap()[:, kc * 128 : (kc + 1) * 128, :].rearrange(
                            "pp part col -> part pp col"
                        ),
                    )
                nc.gpsimd.dma_start(
                    xT_sb[:, kc, :], xT_d.ap()[kc * 128 : (kc + 1) * 128, :]
                )

            vT = [ph1.tile([128, L], F16, tag=f"vT{p}", name=f"vT{p}") for p in range(2)]
            for p in range(2):
                for ib in range(4):
                    isl = bass.ts(ib, 512)
                    for name in ("q", "k", "v"):
                        ps_t = ph1ps.tile([128, 512], F32, tag="qkv")
                        for kc in range(8):
                            nc.tensor.matmul(
                                ps_t[:, :],
                                lhsT=w_sb[name][:, p, kc, :],
                                rhs=xT_sb[:, kc, isl],
                                start=(kc == 0),
                                stop=(kc == 7),
                            )
                        if name == "q":
                            nc.vector.tensor_scalar_add(qT[p][:, isl], ps_t[:, :], bq_sb[:, p : p + 1])
                        elif name == "k":
                            nc.vector.tensor_scalar_add(kT[p][:, isl], ps_t[:, :], bk_sb[:, p : p + 1])
                        else:
                            nc.vector.tensor_copy(vT[p][:, isl], ps_t[:, :])
            # wide xbar transposes vT -> vperm (same primitive as P^T below)
            for p in range(2):
                for jb in range(4):
                    nc.sync.dma_start_transpose(
                        vperm[p][:, jb, :, :], vT[p][:, bass.ts(jb, 512)]
                    )

        # ---------- phases 2+3 per head-pair ----------
        with tc.tile_pool(name="p23", bufs=1) as p23, \
             tc.tile_pool(name="p2tmp", bufs=2) as p2tmp, \
             tc.tile_pool(name="p3qk0", bufs=2, space="PSUM") as p3qk0, \
             tc.tile_pool(name="p3qk1", bufs=2, space="PSUM") as p3qk1, \
             tc.tile_pool(name="p3y", bufs=2, space="PSUM") as p3y, \
             tc.tile_pool(name="p3pk", bufs=2) as p3pk, \
             tc.tile_pool(name="p3es", bufs=3) as p3es, \
             tc.tile_pool(name="p3pw", bufs=2) as p3pw, \
             tc.tile_pool(name="p3pt", bufs=2) as p3pt, \
             tc.tile_pool(name="p3rs", bufs=2) as p3rs:
            erT_sb = p23.tile([128, L], F32R)
            nc.gpsimd.dma_start(erT_sb[0:HS, :], erT_d.ap())
            nc.gpsimd.dma_start(erT_sb[HS:128, :], erT_d.ap())
            p3qk = [p3qk0, p3qk1]

            pending = []

            def _emit_attv(item):
                pp_, IB_, njb_, ptjb_, rb_ = item
                for s_ in range(2):
                    ps_y = p3y.tile([64, 512], F32, tag=f"y{s_}", name=f"yps{s_}")
                    nmm = 4 * njb_
                    m_ = 0
                    for jb_ in range(njb_):
                        for c_ in range(4):
                            nc.tensor.matmul(
                                ps_y[:, :],
                                lhsT=vperm[pp_][:, jb_, c_, 64 * s_ : 64 * s_ + 64],
                                rhs=ptjb_[(s_, jb_)][:, c_, :],
                                start=(m_ == 0),
                                stop=(m_ == nmm - 1),
                            )
                            m_ += 1
                    nc.vector.scalar_tensor_tensor(
                        out=yT[pp_][64 * s_ : 64 * s_ + 64, bass.ts(IB_, 512)],
                        in0=ps_y[:, :],
                        scalar=1.0,
                        in1=rb_[s_][:, :],
                        op0=mybir.AluOpType.bypass,
                        op1=MULT,
                    )

            for p in range(2):
                for IB in range(4):
                    # ----- phase 2: EQr stripes for this IB (rows [512*IB, 512*IB+512)) -----
                    for ib in range(4 * IB, 4 * IB + 4):
                        i0 = ib * 128
                        mlo = (L - 128 - i0) // 512
                        for s in range(2):
                            h = 2 * p + s
                            sl = slice(64 * s, 64 * s + 64)
                            eq = p2tmp.tile([128, 2048], F16, tag=f"eq{s}", name=f"eq{s}")
                            for mb in range(mlo, 4):
                                ps_t = p3qk[s].tile([128, 512], F32, tag=f"qk{s}", name=f"eqrps{s}")
                                nc.tensor.matmul(
                                    ps_t[:, :],
                                    lhsT=qT[p][sl, i0 : i0 + 128],
                                    rhs=erT_sb[sl, bass.ts(mb, 512)],
                                    start=True,
                                    stop=True,
                                    tile_position=(64 * s, 0),
                                )
                                nc.scalar.activation(
                                    eq[:, bass.ts(mb, 512)],
                                    ps_t[:, :],
                                    EXP,
                                    scale=SCALE,
                                )
                            nc.gpsimd.dma_start(
                                eqr_d[h].ap()[i0 : i0 + 128, mlo * 512 : 2048],
                                eq[:, mlo * 512 : 2048],
                            )
                    # ----- phase 3 for this IB -----
                    njb = IB + 1
                    W = njb * 512
                    ptjb = {}
                    rb = {}
                    for s in range(2):
                        h = 2 * p + s
                        sl = slice(64 * s, 64 * s + 64)
                        rsall = p3rs.tile([128, 4], F32, tag=f"rs{s}", name=f"rs{s}")
                        for ib4 in range(4):
                            i0 = 512 * IB + 128 * ib4
                            # skew read first: the DMA only depends on the
                            # already-written EQr stripe, so let it start early
                            es = p3es.tile([128, 2048], F16, tag=f"es{s}", name=f"es{s}")
                            src = bass.AP(
                                tensor=eqr_d[h],
                                offset=(L - 1) + i0 * (LP - 1),
                                ap=[[LP - 1, 128], [1, W]],
                            )
                            nc.gpsimd.dma_start(es[:, 0:W], src)
                            pk = p3pk.tile([128, 2048], F16, tag=f"pk{s}", name=f"pk{s}")
                            for jb in range(njb):
                                ps_t = p3qk[s].tile([128, 512], F32, tag=f"qk{s}", name=f"qkps{s}")
                                nc.tensor.matmul(
                                    ps_t[:, :],
                                    lhsT=qT[p][sl, i0 : i0 + 128],
                                    rhs=kT[p][sl, bass.ts(jb, 512)],
                                    start=True,
                                    stop=True,
                                    tile_position=(64 * s, 0),
                                )
                                nc.scalar.activation(
                                    pk[:, bass.ts(jb, 512)],
                                    ps_t[:, :],
                                    EXP,
                                    scale=SCALE,
                                )
                            # combine + full row-sum in one DVE op (P stays unnormalized)
                            pw = p3pw.tile([128, 2048], F16, tag=f"pw{s}", name=f"pw{s}")
                            nc.vector.scalar_tensor_tensor(
                                out=pw[:, 0:W],
                                in0=pk[:, 0:W],
                                scalar=1.0,
                                in1=es[:, 0:W],
                                op0=MULT,
                                op1=MULT,
                                accum_out=rsall[:, ib4 : ib4 + 1],
                            )
                            # wide xbar transposes into the interleaved P^T tiles
                            for jb in range(njb):
                                key = (s, jb)
                                if key not in ptjb:
                                    ptjb[key] = p3pt.tile(
                                        [128, 4, 512], F16, tag=f"pt{s}_{jb}", name=f"pt{s}_{jb}"
                                    )
                                nc.sync.dma_start_transpose(
                                    ptjb[key][:, :, ib4 * 128 : ib4 * 128 + 128],
                                    pw[:, bass.ts(jb, 512)],
                                )
                        # 1/rowsum -> [1, i] layout -> broadcast to 64 partitions
                        riall = p3rs.tile([128, 4], F32, tag=f"ri{s}", name=f"ri{s}")
                        nc.vector.reciprocal(riall[:, :], rsall[:, :])
                        rbt = p3rs.tile([64, 512], F32, tag=f"rb{s}", name=f"rb{s}")
                        for ib4 in range(4):
                            rT1 = p3qk[s].tile([1, 128], F32, tag=f"qk{s}", name=f"rT{s}_{ib4}")
                            nc.tensor.transpose(
                                rT1[:, :], riall[:, ib4 : ib4 + 1], ident[:, :]
                            )
                            rsb1 = p3rs.tile([1, 128], F32, tag=f"rsb{s}", name=f"rsb{s}_{ib4}")
                            nc.vector.tensor_copy(rsb1[:, :], rT1[0:1, :])
                            nc.gpsimd.partition_broadcast(
                                rbt[:, bass.ts(ib4, 128)], rsb1[0:1, :]
                            )
                        rb[s] = rbt
                    # software-pipelined att@v: emit for the PREVIOUS group
                    pending.append((p, IB, njb, ptjb, rb))
                    if len(pending) > 1:
                        _emit_attv(pending.pop(0))

            while pending:
                _emit_attv(pending.pop(0))

        # ---------- phase 4: out_partial = y @ Wproj ----------
        with tc.tile_pool(name="p4ps", bufs=3, space="PSUM") as p4ps, \
             tc.tile_pool(name="p4tmp", bufs=3) as p4tmp:
            for i128 in range(16):
                for eb in range(2):
                    ps_o = p4ps.tile([128, 512], F32, tag="proj")
                    for p in range(2):
                        nc.tensor.matmul(
                            ps_o[:, :],
                            lhsT=yT[p][:, bass.ts(i128, 128)],
                            rhs=wproj_sb[:, p, bass.ts(eb, 512)],
                            start=(p == 0),
                            stop=(p == 1),
                        )
                    o_t = p4tmp.tile([128, 512], F16, tag="out")
                    nc.vector.tensor_copy(o_t[:, :], ps_o[:, :])
                    nc.sync.dma_start(
                        out_d.ap()[bass.ts(i128, 128), bass.ts(eb, 512)], o_t[:, :]
                    )

    nc.compile()
    return nc


_NC_CACHE = None
RUN_KWARGS = {}
LAST_RESULTS = None
LAST_IN_MAPS = None


def _get_program():
    global _NC_CACHE
    if _NC_CACHE is None:
        _NC_CACHE = _build_program()
    return _NC_CACHE


def kernel(x, Wqkv, bqkv, Wproj, bproj, Er):
    x = np.asarray(x, dtype=np.float32)
    Wqkv = np.asarray(Wqkv, dtype=np.float32)
    bqkv = np.asarray(bqkv, dtype=np.float32)
    Wproj = np.asarray(Wproj, dtype=np.float32)
    bproj = np.asarray(bproj, dtype=np.float32)
    Er = np.asarray(Er, dtype=np.float32)

    nc = _get_program()

    start = BLOCK_SIZE - L
    erT = np.ascontiguousarray(Er[start:, :].T)  # [HS, L]

    in_maps = []
    for c in range(N_CORES):
        b = c // 4
        h0 = HPC * (c % 4)
        xT = np.ascontiguousarray(x[b].T).astype(np.float16)  # [D, L]
        wq = np.empty((2, D, 128), np.float16)
        wk = np.empty((2, D, 128), np.float16)
        wv = np.empty((2, D, 128), np.float16)
        bq = np.empty((2, 128, 1), np.float32)
        bk = np.empty((2, 128, 1), np.float32)
        wproj = np.empty((2, 128, D), np.float16)
        for p in range(2):
            c0 = (h0 + 2 * p) * HS  # first head dim offset of the pair
            wq[p] = Wqkv[:, c0 : c0 + 128]
            wk[p] = Wqkv[:, D + c0 : D + c0 + 128]
            wv[p] = Wqkv[:, 2 * D + c0 : 2 * D + c0 + 128]
            bq[p, :, 0] = bqkv[c0 : c0 + 128]
            bk[p, :, 0] = bqkv[D + c0 : D + c0 + 128]
            wproj[p] = Wproj[c0 : c0 + 128, :].astype(np.float16)
        in_maps.append(
            {
                "xT": xT,
                "wq": wq,
                "wk": wk,
                "wv": wv,
                "bq": bq,
                "bk": bk,
                "erT": erT,
                "wproj": wproj,
            }
        )

    global LAST_RESULTS, LAST_IN_MAPS
    LAST_IN_MAPS = in_maps
    res = run_bass_kernel_spmd(nc, in_maps, core_ids=list(range(N_CORES)), **RUN_KWARGS)
    LAST_RESULTS = res

    # host gather: sum row-parallel partials per batch; fold bv@Wproj + bproj
    bv = bqkv[2 * D :]
    bias_vec = bv @ Wproj + bproj  # exact: softmax rows sum to 1
    out = np.zeros((B, L, D), np.float32)
    for c in range(N_CORES):
        out[c // 4] += res.results[c]["out"].astype(np.float32)
    out += bias_vec[None, None, :]
    return out


# revision 28
# speedup vs baseline: 3.0816x; 1.1926x over previous
"""Causal self-attention with Music-Transformer relative position, on 8 TRN2 cores.

Self-contained: takes FULL inputs, shards internally, returns FULL output.

Sharding: core c -> batch b = c // 4, heads [4*(c%4), 4*(c%4)+4)  (2 head-pairs).
Each core computes its qkv column-slice, attention for its 4 heads, and a
row-parallel partial of the output projection. Host sums the 8 partials.

Key tricks:
- Skew: Srel[i, j] = QEr[i, L-1-i+j], so with a padded row-major DRAM buffer
  EQr[L, LP] (LP = L+512, pad = 0) holding exp(QEr/8), a single strided DMA
  (row stride LP-1) yields skew(exp(QEr/8)); exp commutes with the skew:
      P = exp((QK + Srel)/8) = exp(QK/8) * skew(exp(QEr/8))
  and the zero pad makes the causal mask free.
- P^T for the P@V matmul comes from wide xbar DMA-transposes ([128, 512] ->
  [128, 4, 128]) whose j-interleave is whatever the hardware produces —
  v is pre-transposed with the *same* primitive, so both operands of the
  contraction share one permutation of j and the matmul result is unchanged.
- Row-paired K=64 matmuls (two heads on PE row-groups (0,0)/(64,0)).
"""

import numpy as np
from contextlib import ExitStack

import concourse.bass as bass
import concourse.tile as tile
from concourse import mybir, bacc
from concourse.bass_utils import run_bass_kernel_spmd

F32 = mybir.dt.float32
F32R = mybir.dt.float32r
F16 = mybir.dt.float16

B, L, D = 2, 2048, 1024
NH, HS = 16, 64
BLOCK_SIZE = 2048
SCALE = 1.0 / 8.0  # 1/sqrt(HS)
LP = L + 512       # padded EQr row length
N_CORES = 8
HPC = 4            # heads per core (2 pairs)

EXP = mybir.ActivationFunctionType.Exp
COPY = mybir.ActivationFunctionType.Copy
MULT = mybir.AluOpType.mult


def _build_program():
    nc = bacc.Bacc("TRN2", target_bir_lowering=False, debug=False)

    # ---- per-core inputs (f32r dtypes map to np.float32 on the host side) ----
    xT_d = nc.dram_tensor("xT", [D, L], F16, kind="ExternalInput")
    wq_d = nc.dram_tensor("wq", [2, D, 128], F16, kind="ExternalInput")
    wk_d = nc.dram_tensor("wk", [2, D, 128], F16, kind="ExternalInput")
    wv_d = nc.dram_tensor("wv", [2, D, 128], F16, kind="ExternalInput")
    bq_d = nc.dram_tensor("bq", [2, 128, 1], F32, kind="ExternalInput")
    bk_d = nc.dram_tensor("bk", [2, 128, 1], F32, kind="ExternalInput")
    erT_d = nc.dram_tensor("erT", [HS, L], F32R, kind="ExternalInput")
    wproj_d = nc.dram_tensor("wproj", [2, 128, D], F16, kind="ExternalInput")
    out_d = nc.dram_tensor("out", [L, D], F16, kind="ExternalOutput")

    # per-head padded EQr scratch (separate tensors => fine-grained deps)
    eqr_d = [nc.dram_tensor(f"eqr{h}", [L, LP], F16, kind="Internal") for h in range(HPC)]

    with tile.TileContext(nc) as tc, ExitStack() as ctx:
        # ---------- persistent tiles ----------
        persist = ctx.enter_context(tc.tile_pool(name="persist", bufs=1))
        qT = [persist.tile([128, L], F32R, tag=f"qT{p}", name=f"qT{p}") for p in range(2)]
        kT = [persist.tile([128, L], F32R, tag=f"kT{p}", name=f"kT{p}") for p in range(2)]
        # v_perm[p][pp, jb, c, d'] = V[pair p][j = xbar-perm(jb, pp, c), d']
        vperm = [persist.tile([128, 4, 4, 128], F16, tag=f"vperm{p}", name=f"vperm{p}")
                 for p in range(2)]
        yT = [persist.tile([128, L], F16, tag=f"yT{p}", name=f"yT{p}") for p in range(2)]
        wproj_sb = persist.tile([128, 2, D], F16, tag="wproj")
        bq_sb = persist.tile([128, 2], F32, tag="bq")
        bk_sb = persist.tile([128, 2], F32, tag="bk")
        zero16 = persist.tile([128, 512], F16, tag="zero16")
        nc.vector.memset(zero16[:, :], 0.0)
        ident = persist.tile([128, 128], F32, tag="ident")
        from concourse.masks import make_identity
        make_identity(nc, ident[:, :])
        for p in range(2):
            nc.gpsimd.dma_start(wproj_sb[:, p, :], wproj_d.ap()[p])
            nc.gpsimd.dma_start(bq_sb[:, p : p + 1], bq_d.ap()[p])
            nc.gpsimd.dma_start(bk_sb[:, p : p + 1], bk_d.ap()[p])

        # pad region of every eqr
        for h in range(HPC):
            for g in range(16):
                nc.sync.dma_start(eqr_d[h].ap()[g * 128 : (g + 1) * 128, L:LP], zero16[:, :])

        # ---------- phase 1: qkv projection ----------
        with tc.tile_pool(name="ph1", bufs=1) as ph1, \
             tc.tile_pool(name="ph1ps", bufs=3, space="PSUM") as ph1ps, \
             tc.tile_pool(name="ph1tmp", bufs=3) as ph1tmp:
            xT_sb = ph1.tile([128, 8, L], F16)
            w_sb = {}
            for name, d_ in (("q", wq_d), ("k", wk_d), ("v", wv_d)):
                w_sb[name] = ph1.tile([128, 2, 8, 128], F16, tag=f"w{name}", name=f"w{name}sb")
            # per-kc loads: fine-grained deps let the first matmuls start early
            for kc in range(8):
                for name, d_ in (("q", wq_d), ("k", wk_d), ("v", wv_d)):
                    nc.gpsimd.dma_start(
                        w_sb[name][:, :, kc, :],
                        d_.ap()[:, kc * 128 : (kc + 1) * 128, :].rearrange(
                            "pp part col -> part pp col"
                        ),
                    )
                nc.gpsimd.dma_start(
                    xT_sb[:, kc, :], xT_d.ap()[kc * 128 : (kc + 1) * 128, :]
                )

            vT = [ph1.tile([128, L], F16, tag=f"vT{p}", name=f"vT{p}") for p in range(2)]
            for p in range(2):
                for ib in range(4):
                    isl = bass.ts(ib, 512)
                    for name in ("q", "k", "v"):
                        ps_t = ph1ps.tile([128, 512], F32, tag="qkv")
                        for kc in range(8):
                            nc.tensor.matmul(
                                ps_t[:, :],
                                lhsT=w_sb[name][:, p, kc, :],
                                rhs=xT_sb[:, kc, isl],
                                start=(kc == 0),
                                stop=(kc == 7),
                            )
                        if name == "q":
                            nc.vector.tensor_scalar_add(qT[p][:, isl], ps_t[:, :], bq_sb[:, p : p + 1])
                        elif name == "k":
                            nc.vector.tensor_scalar_add(kT[p][:, isl], ps_t[:, :], bk_sb[:, p : p + 1])
                        else:
                            nc.vector.tensor_copy(vT[p][:, isl], ps_t[:, :])
            # wide xbar transposes vT -> vperm (same primitive as P^T below)
            for p in range(2):
                for jb in range(4):
                    nc.sync.dma_start_transpose(
                        vperm[p][:, jb, :, :], vT[p][:, bass.ts(jb, 512)]
                    )

        # ---------- phases 2+3 per head-pair ----------
        with tc.tile_pool(name="p23", bufs=1) as p23, \
             tc.tile_pool(name="p2tmp", bufs=2) as p2tmp, \
             tc.tile_pool(name="p3qk0", bufs=2, space="PSUM") as p3qk0, \
             tc.tile_pool(name="p3qk1", bufs=2, space="PSUM") as p3qk1, \
             tc.tile_pool(name="p3y", bufs=2, space="PSUM") as p3y, \
             tc.tile_pool(name="p3pk", bufs=2) as p3pk, \
             tc.tile_pool(name="p3es", bufs=3) as p3es, \
             tc.tile_pool(name="p3pw", bufs=2) as p3pw, \
             tc.tile_pool(name="p3pt", bufs=2) as p3pt, \
             tc.tile_pool(name="p3rs", bufs=2) as p3rs:
            erT_sb = p23.tile([128, L], F32R)
            nc.gpsimd.dma_start(erT_sb[0:HS, :], erT_d.ap())
            nc.gpsimd.dma_start(erT_sb[HS:128, :], erT_d.ap())
            p3qk = [p3qk0, p3qk1]

            pending = []

            def _emit_attv(item):
                pp_, IB_, njb_, ptjb_, rb_ = item
                for s_ in range(2):
                    ps_y = p3y.tile([64, 512], F32, tag=f"y{s_}", name=f"yps{s_}")
                    nmm = 4 * njb_
                    m_ = 0
                    for jb_ in range(njb_):
                        for c_ in range(4):
                            nc.tensor.matmul(
                                ps_y[:, :],
                                lhsT=vperm[pp_][:, jb_, c_, 64 * s_ : 64 * s_ + 64],
                                rhs=ptjb_[(s_, jb_)][:, c_, :],
                                start=(m_ == 0),
                                stop=(m_ == nmm - 1),
                            )
                            m_ += 1
                    nc.vector.scalar_tensor_tensor(
                        out=yT[pp_][64 * s_ : 64 * s_ + 64, bass.ts(IB_, 512)],
                        in0=ps_y[:, :],
                        scalar=1.0,
                        in1=rb_[s_][:, :],
                        op0=mybir.AluOpType.bypass,
                        op1=MULT,
                    )

            for p in range(2):
                for IB in range(4):
                    # ----- phase 2: EQr stripes for this IB (rows [512*IB, 512*IB+512)) -----
                    for ib in range(4 * IB, 4 * IB + 4):
                        i0 = ib * 128
                        mlo = (L - 128 - i0) // 512
                        for s in range(2):
                            h = 2 * p + s
                            sl = slice(64 * s, 64 * s + 64)
                            eq = p2tmp.tile([128, 2048], F16, tag=f"eq{s}", name=f"eq{s}")
                            for mb in range(mlo, 4):
                                ps_t = p3qk[s].tile([128, 512], F32, tag=f"qk{s}", name=f"eqrps{s}")
                                nc.tensor.matmul(
                                    ps_t[:, :],
                                    lhsT=qT[p][sl, i0 : i0 + 128],
                                    rhs=erT_sb[sl, bass.ts(mb, 512)],
                                    start=True,
                                    stop=True,
                                    tile_position=(64 * s, 0),
                                )
                                nc.scalar.activation(
                                    eq[:, bass.ts(mb, 512)],
                                    ps_t[:, :],
                                    EXP,
                                    scale=SCALE,
                                )
                            nc.gpsimd.dma_start(
                                eqr_d[h].ap()[i0 : i0 + 128, mlo * 512 : 2048],
                                eq[:, mlo * 512 : 2048],
                            )
                    # ----- phase 3 for this IB -----
                    njb = IB + 1
                    W = njb * 512
                    ptjb = {}
                    rb = {}
                    for s in range(2):
                        h = 2 * p + s
                        sl = slice(64 * s, 64 * s + 64)
                        rsall = p3rs.tile([128, 4], F32, tag=f"rs{s}", name=f"rs{s}")
                        for ib4 in range(4):
                            i0 = 512 * IB + 128 * ib4
                            # skew read first: the DMA only depends on the
                            # already-written EQr stripe, so let it start early
                            es = p3es.tile([128, 2048], F16, tag=f"es{s}", name=f"es{s}")
                            src = bass.AP(
                                tensor=eqr_d[h],
                                offset=(L - 1) + i0 * (LP - 1),
                                ap=[[LP - 1, 128], [1, W]],
                            )
                            nc.gpsimd.dma_start(es[:, 0:W], src)
                            pk = p3pk.tile([128, 2048], F16, tag=f"pk{s}", name=f"pk{s}")
                            for jb in range(njb):
                                ps_t = p3qk[s].tile([128, 512], F32, tag=f"qk{s}", name=f"qkps{s}")
                                nc.tensor.matmul(
                                    ps_t[:, :],
                                    lhsT=qT[p][sl, i0 : i0 + 128],
                                    rhs=kT[p][sl, bass.ts(jb, 512)],
                                    start=True,
                                    stop=True,
                                    tile_position=(64 * s, 0),
                                )
                                nc.scalar.activation(
                                    pk[:, bass.ts(jb, 512)],
                                    ps_t[:, :],
                                    EXP,
                                    scale=SCALE,
                                )
                            # combine + full row-sum in one DVE op (P stays unnormalized)
                            pw = p3pw.tile([128, 2048], F16, tag=f"pw{s}", name=f"pw{s}")
                            nc.vector.scalar_tensor_tensor(
                                out=pw[:, 0:W],
                                in0=pk[:, 0:W],
                                scalar=1.0,
                                in1=es[:, 0:W],
                                op0=MULT,
                                op1=MULT,
                                accum_out=rsall[:, ib4 : ib4 + 1],
                            )
                            # wide xbar transposes into the interleaved P^T tiles
                            for jb in range(njb):
                                key = (s, jb)
                                if key not in ptjb:
                                    ptjb[key] = p3pt.tile(
                                        [128, 4, 512], F16, tag=f"pt{s}_{jb}", name=f"pt{s}_{jb}"
                                    )
                                nc.sync.dma_start_transpose(
                                    ptjb[key][:, :, ib4 * 128 : ib4 * 128 + 128],
                                    pw[:, bass.ts(jb, 512)],
                                )
                        # 1/rowsum -> [1, i] layout -> broadcast to 64 partitions
                        riall = p3rs.tile([128, 4], F32, tag=f"ri{s}", name=f"ri{s}")
                        nc.vector.reciprocal(riall[:, :], rsall[:, :])
                        rbt = p3rs.tile([64, 512], F32, tag=f"rb{s}", name=f"rb{s}")
                        for ib4 in range(4):
                            rT1 = p3qk[s].tile([1, 128], F32, tag=f"qk{s}", name=f"rT{s}_{ib4}")
                            nc.tensor.transpose(
                                rT1[:, :], riall[:, ib4 : ib4 + 1], ident[:, :]
                            )
                            rsb1 = p3rs.tile([1, 128], F32, tag=f"rsb{s}", name=f"rsb{s}_{ib4}")
                            nc.vector.tensor_copy(rsb1[:, :], rT1[0:1, :])
                            nc.gpsimd.partition_broadcast(
                                rbt[:, bass.ts(ib4, 128)], rsb1[0:1, :]
                            )
                        rb[s] = rbt
                    # software-pipelined att@v: emit for the PREVIOUS group
                    pending.append((p, IB, njb, ptjb, rb))
                    if len(pending) > 1:
                        _emit_attv(pending.pop(0))

            while pending:
                _emit_attv(pending.pop(0))

        # ---------- phase 4: out_partial = y @ Wproj ----------
        with tc.tile_pool(name="p4ps", bufs=3, space="PSUM") as p4ps, \
             tc.tile_pool(name="p4tmp", bufs=3) as p4tmp:
            for i128 in range(16):
                for eb in range(2):
                    ps_o = p4ps.tile([128, 512], F32, tag="proj")
                    for p in range(2):
                        nc.tensor.matmul(
                            ps_o[:, :],
                            lhsT=yT[p][:, bass.ts(i128, 128)],
                            rhs=wproj_sb[:, p, bass.ts(eb, 512)],
                            start=(p == 0),
                            stop=(p == 1),
                        )
                    o_t = p4tmp.tile([128, 512], F16, tag="out")
                    nc.vector.tensor_copy(o_t[:, :], ps_o[:, :])
                    nc.sync.dma_start(
                        out_d.ap()[bass.ts(i128, 128), bass.ts(eb, 512)], o_t[:, :]
                    )

    nc.compile()
    return nc


_NC_CACHE = None
RUN_KWARGS = {}
LAST_RESULTS = None
LAST_IN_MAPS = None


def _get_program():
    global _NC_CACHE
    if _NC_CACHE is None:
        _NC_CACHE = _build_program()
    return _NC_CACHE


def kernel(x, Wqkv, bqkv, Wproj, bproj, Er):
    x = np.asarray(x, dtype=np.float32)
    Wqkv = np.asarray(Wqkv, dtype=np.float32)
    bqkv = np.asarray(bqkv, dtype=np.float32)
    Wproj = np.asarray(Wproj, dtype=np.float32)
    bproj = np.asarray(bproj, dtype=np.float32)
    Er = np.asarray(Er, dtype=np.float32)

    nc = _get_program()

    start = BLOCK_SIZE - L
    erT = np.ascontiguousarray(Er[start:, :].T)  # [HS, L]

    in_maps = []
    for c in range(N_CORES):
        b = c // 4
        h0 = HPC * (c % 4)
        xT = np.ascontiguousarray(x[b].T).astype(np.float16)  # [D, L]
        wq = np.empty((2, D, 128), np.float16)
        wk = np.empty((2, D, 128), np.float16)
        wv = np.empty((2, D, 128), np.float16)
        bq = np.empty((2, 128, 1), np.float32)
        bk = np.empty((2, 128, 1), np.float32)
        wproj = np.empty((2, 128, D), np.float16)
        for p in range(2):
            c0 = (h0 + 2 * p) * HS  # first head dim offset of the pair
            wq[p] = Wqkv[:, c0 : c0 + 128]
            wk[p] = Wqkv[:, D + c0 : D + c0 + 128]
            wv[p] = Wqkv[:, 2 * D + c0 : 2 * D + c0 + 128]
            bq[p, :, 0] = bqkv[c0 : c0 + 128]
            bk[p, :, 0] = bqkv[D + c0 : D + c0 + 128]
            wproj[p] = Wproj[c0 : c0 + 128, :].astype(np.float16)
        in_maps.append(
            {
                "xT": xT,
                "wq": wq,
                "wk": wk,
                "wv": wv,
                "bq": bq,
                "bk": bk,
                "erT": erT,
                "wproj": wproj,
            }
        )

    global LAST_RESULTS, LAST_IN_MAPS
    LAST_IN_MAPS = in_maps
    res = run_bass_kernel_spmd(nc, in_maps, core_ids=list(range(N_CORES)), **RUN_KWARGS)
    LAST_RESULTS = res

    # host gather: sum row-parallel partials per batch; fold bv@Wproj + bproj
    bv = bqkv[2 * D :]
    bias_vec = bv @ Wproj + bproj  # exact: softmax rows sum to 1
    out = np.zeros((B, L, D), np.float32)
    for c in range(N_CORES):
        out[c // 4] += res.results[c]["out"].astype(np.float32)
    out += bias_vec[None, None, :]
    return out
